# revision 41
# baseline (speedup 1.0000x reference)
"""AttentionSubsample Trainium2 kernel: 8-core data-parallel over batch.

Layout strategy (per core, 4 batch elements):
  - All matmuls contract over the SBUF partition dim.
  - kv "k" part + q computed feature-major [feat, tok]; BN folded into weights
    (scale) and eviction bias (shift, per-partition).
  - v computed token-major [tok, feat] so attn@v needs no transpose; its BN
    shift is applied after attention via the softmax-denominator identity.
  - scores computed as s.T [ktok, qtok] per (head, chunk); bias applied as
    exp(s)*exp(bias) with a host-gathered exp-bias table; softmax sum over
    ktok obtained by appending a ones-column to v (row 64 of attn@v output).
  - output projection computed token-major directly (lhsT = hardswish acts),
    BN shift via a K=1 ones-row matmul, so the final store is contiguous.
  - matmul cost is set by the MOVING operand dtype: fp32 = 4 cyc/row, fp32r =
    1 cyc/row only at moving >= 256, bf16 = 1 cyc/row always. So the big
    >=256-moving matmuls (k, v, proj, batched q) keep fp32r moving operands,
    while the 196-moving ones (scores, attn@v, recip-broadcast) use bf16
    moving operands (q_sb, texp, rec) with full-precision fp32r stationaries.
  - exp-bias table is bf16 and loaded once per head-group (not per batch);
    the bias multiply runs on DVE in bf16 (2x mode); v/acc PSUM evictions run
    on the otherwise-idle GPSIMD (Pool) engine.
"""

import sys

sys.path.insert(0, "/opt/trn_rl_repo")

from contextlib import ExitStack

import numpy as np
import ml_dtypes

import concourse.bass as bass
import concourse.tile as tile
from concourse import bacc
from concourse import mybir
from concourse.bass_utils import run_bass_kernel_spmd

F32 = mybir.dt.float32
F32R = mybir.dt.float32r
BF16 = mybir.dt.bfloat16
ALU = mybir.AluOpType
AF = mybir.ActivationFunctionType

B, N, NQ, IN, H, KD, D, OUT = 32, 784, 196, 384, 16, 32, 64, 512
HID, DH = 1536, 1024
RES, RES_, STRIDE = 28, 14, 2
SCALE = KD ** -0.5
EPS = 1e-5
NCORES = 8
BC = B // NCORES          # 4 batch elems per core
C, MC = 7, 112            # key-token chunks: 7 x 112 = 784
G, HG = 2, 8              # 2 head-groups of 8 heads

TRACE = False
LAST_RESULTS = None

_NC_CACHE = None


def _build_nc():
    nc = bacc.Bacc("TRN2", target_bir_lowering=False, debug=False,
                   num_devices=NCORES)

    xT = nc.dram_tensor("xT", [BC, IN, N], BF16, kind="ExternalInput").ap()
    xsT = nc.dram_tensor("xsT", [BC, IN, NQ], BF16, kind="ExternalInput").ap()
    wk = nc.dram_tensor("wk", [IN, 512], BF16, kind="ExternalInput").ap()
    wv = nc.dram_tensor("wv", [IN, DH], BF16, kind="ExternalInput").ap()
    wq = nc.dram_tensor("wq", [IN, 512], BF16, kind="ExternalInput").ap()
    wp = nc.dram_tensor("wp", [DH, OUT], BF16, kind="ExternalInput").ap()
    shk = nc.dram_tensor("shk", [128, 4], F32, kind="ExternalInput").ap()
    shq = nc.dram_tensor("shq", [128, 4], F32, kind="ExternalInput").ap()
    shv = nc.dram_tensor("shv", [128, 8], F32, kind="ExternalInput").ap()
    shp = nc.dram_tensor("shp", [1, OUT], F32R, kind="ExternalInput").ap()
    ebias = nc.dram_tensor("ebias", [MC, G, C, HG, NQ], BF16,
                           kind="ExternalInput").ap()
    seld = nc.dram_tensor("seld", [16, 8, 128], BF16, kind="ExternalInput").ap()
    out = nc.dram_tensor("out", [BC, NQ, OUT], F32, kind="ExternalOutput").ap()

    with tile.TileContext(nc) as tc, ExitStack() as ctx:
        singles = ctx.enter_context(tc.tile_pool(name="singles", bufs=1))
        biasp = ctx.enter_context(tc.tile_pool(name="biasp", bufs=2))
        kqp = ctx.enter_context(tc.tile_pool(name="kqp", bufs=2))
        vp = ctx.enter_context(tc.tile_pool(name="vp", bufs=2))
        texpp = ctx.enter_context(tc.tile_pool(name="texpp", bufs=3))
        accp = ctx.enter_context(tc.tile_pool(name="accp", bufs=1))
        tmpp = ctx.enter_context(tc.tile_pool(name="tmpp", bufs=2))
        hswp = ctx.enter_context(tc.tile_pool(name="hswp", bufs=1))
        finp = ctx.enter_context(tc.tile_pool(name="finp", bufs=2))
        mmp = ctx.enter_context(tc.tile_pool(name="mmp", bufs=2, space="PSUM"))
        scp = ctx.enter_context(tc.tile_pool(name="scp", bufs=2, space="PSUM"))
        opp = ctx.enter_context(tc.tile_pool(name="opp", bufs=2, space="PSUM"))

        # --- persistent SBUF, DMAs ordered by first use (the cost model
        # serializes all DMA traffic, so queue order gates the startup) ---
        # DMAs ordered by first compute use: k(b0) inputs (Act queue, so the
        # first matmuls wait on a semaphore covering just these), then v/q
        # inputs on the SP queue.
        wk_sb = singles.tile([128, 3, 512], BF16)
        nc.scalar.dma_start(wk_sb, wk.rearrange("(c p) n -> p c n", p=128))
        shk_sb = singles.tile([128, 4], F32)
        nc.scalar.dma_start(shk_sb, shk)
        # x for all 4 batch elems, loaded once and reused by both head groups
        xtb4 = [singles.tile([128, 3, N], BF16, name=f"xtb{b}")
                for b in range(BC)]
        nc.scalar.dma_start(xtb4[0], xT[0].rearrange("(c p) n -> p c n", p=128))
        wv_sb = singles.tile([128, 3, DH], BF16)
        nc.sync.dma_start(wv_sb, wv.rearrange("(c p) n -> p c n", p=128))
        wq_sb = singles.tile([128, 3, 512], BF16)
        nc.sync.dma_start(wq_sb, wq.rearrange("(c p) n -> p c n", p=128))
        shq_sb = singles.tile([128, 4], F32)
        nc.sync.dma_start(shq_sb, shq)
        # subsampled x for all 4 batch elems, loaded once
        xstb = singles.tile([128, 3, BC, NQ], BF16)
        for bb in range(BC):
            nc.sync.dma_start(xstb[:, :, bb, :],
                              xsT[bb].rearrange("(c p) n -> p c n", p=128))
        ones1 = singles.tile([1, 128], F32R)
        nc.gpsimd.memset(ones1.bitcast(F32), 1.0)
        # output-phase constants are DMA'd after the attention loop starts;
        # declared here, loaded right before the output phase
        wp_sb = singles.tile([128, 8, OUT], BF16)
        shv_sb = singles.tile([128, 8], F32)
        shp_sb = singles.tile([1, OUT], F32R)
        # sel[:, t, :] is a [16, 128] 0/1 matrix: sel[i, t, m] = 1 iff head i
        # feeds output row m of feature-tile t (rows 0-63 <- head 2t, 64-127
        # <- head 2t+1). Used to broadcast softmax reciprocals across rows.
        sel = singles.tile([16, 8, 128], BF16)

        acc = [accp.tile([128, 8, NQ], BF16, name=f"acc{b}") for b in range(BC)]
        # denominator staging: head h=4g+hh -> partition 32*hh, block g
        den = [accp.tile([128, 4, NQ], F32, name=f"den{b}") for b in range(BC)]
        den2 = [accp.tile([16, NQ], F32, name=f"den2{b}") for b in range(BC)]
        recb = [accp.tile([16, NQ], BF16, name=f"recb{b}") for b in range(BC)]

        for g in range(G):
            # exp-bias table for this head group (reused by all 4 batches),
            # c-major [ktok, chunk, head-in-group, qtok] to match texp2d
            ebias_g = biasp.tile([MC, C, HG, NQ], BF16, tag="ebias")
            nc.sync.dma_start(ebias_g, ebias[:, g, :, :, :])
            if g == 0:
                # remaining x batches: needed from (g0, b1) onwards, queued
                # behind the g0 ebias table
                for bb in range(1, BC):
                    nc.sync.dma_start(xtb4[bb],
                                      xT[bb].rearrange("(c p) n -> p c n",
                                                       p=128))

            q_sb = kqp.tile([128, 2, BC, NQ], BF16, tag="q_sb")

            for b in range(BC):
                xtb = xtb4[b]

                # k for this head group: features [256g, 256g+256), feat-major
                k_sb = kqp.tile([128, 2, N], BF16, tag="k_sb")
                for m2 in range(2):
                    for n2 in range(2):
                        pk = mmp.tile([128, 512], F32, tag="mm", name="pk")
                        for kk in range(3):
                            nc.tensor.matmul(
                                pk[:, :392],
                                lhsT=wk_sb[:, kk, 256 * g + 128 * m2:
                                           256 * g + 128 * m2 + 128],
                                rhs=xtb[:, kk, 392 * n2:392 * n2 + 392],
                                start=(kk == 0), stop=(kk == 2))
                        nc.scalar.activation(
                            k_sb[:, m2, 392 * n2:392 * n2 + 392],
                            pk[:, :392], AF.Identity,
                            bias=shk_sb[:, 2 * g + m2:2 * g + m2 + 1])

                # v token-major for this head group (512 features), with an
                # all-ones column appended per head for the softmax denominator
                vtp = vp.tile([MC, C, 8 * 65], BF16, tag="vtp")
                ones_cols = vtp.rearrange("p c (h e) -> p c h e", e=65)[:, :, :, 64:65]
                nc.vector.memset(ones_cols, 1.0)
                for c in range(C):
                    pv = mmp.tile([128, 512], F32, tag="mm", name="pv")
                    for kk in range(3):
                        nc.tensor.matmul(
                            pv[:MC, :],
                            lhsT=xtb[:, kk, MC * c:MC * c + MC],
                            rhs=wv_sb[:, kk, 512 * g:512 * g + 512],
                            start=(kk == 0), stop=(kk == 2))
                    nc.vector.tensor_copy(
                        vtp.rearrange("p c (h e) -> p c h e", e=65)[:, c, :, 0:64],
                        pv[:MC, :].rearrange("p (h d) -> p h d", d=64))

                if b == 0:
                    # q for this head group, all 4 batches: fp32r-rate bf16
                    # moving 392 (2 b per matmul). Emitted after b0's k/v so
                    # the kernel's first matmuls only need wk+xtb0 loaded.
                    for m2 in range(2):
                        for hb in range(2):
                            pq = mmp.tile([128, 512], F32, tag="mm", name="pq")
                            for kk in range(3):
                                nc.tensor.matmul(
                                    pq[:, :392],
                                    lhsT=wq_sb[:, kk, 256 * g + 128 * m2:
                                               256 * g + 128 * m2 + 128],
                                    rhs=xstb[:, kk, 2 * hb:2 * hb + 2, :],
                                    start=(kk == 0), stop=(kk == 2))
                            nc.scalar.activation(
                                q_sb[:, m2, 2 * hb:2 * hb + 2, :],
                                pq[:, :392].rearrange("p (b n) -> p b n", n=NQ),
                                AF.Identity,
                                bias=shq_sb[:, 2 * g + m2:2 * g + m2 + 1])

                # scores + exp + bias-mult, per head pair. texp is c-major
                # [112, chunk, head-of-pair, 196] so each exp output is a
                # contiguous block covering BOTH heads: each score tile packs
                # (chunk, head) as (plane, 196-slot) and one exp drains it.
                for hp in range(HG // 2):
                    texp2d = texpp.tile([MC, C, 2, NQ], BF16, tag="texp")
                    for ct, cs in enumerate(((0, 1), (2, 3), (4, 5), (6,))):
                        # plane j holds head j's chunks (196-slots) so each
                        # PSUM bank sees matmuls from a single PE row band
                        sc = scp.tile([MC, 2, 512], F32, tag="sc",
                                      name=f"sc{ct}")
                        for si, c in enumerate(cs):
                            for j in range(2):
                                hh = 2 * hp + j
                                pb = 32 * (hh % 4)
                                m2 = hh // 4
                                nc.tensor.matmul(
                                    sc[:, j, 196 * si:196 * si + 196],
                                    lhsT=k_sb[pb:pb + 32, m2, MC * c:MC * c + MC],
                                    rhs=q_sb[pb:pb + 32, m2, b, :],
                                    start=True, stop=True,
                                    tile_position=(pb, 0),
                                    skip_group_check=True)
                        if len(cs) == 2:
                            nc.scalar.activation(
                                texp2d[:, cs[0]:cs[0] + 2, :, :],
                                sc[:, :, 0:392].rearrange(
                                    "p a (b q) -> p b a q", q=196), AF.Exp)
                        else:
                            nc.scalar.activation(
                                texp2d[:, 6, :, :],
                                sc[:, :, 0:196], AF.Exp)
                    nc.vector.tensor_tensor(
                        texp2d[:, 0:4, :, :], texp2d[:, 0:4, :, :],
                        ebias_g[:, 0:4, 2 * hp:2 * hp + 2, :], ALU.mult)
                    nc.vector.tensor_tensor(
                        texp2d[:, 4:7, :, :], texp2d[:, 4:7, :, :],
                        ebias_g[:, 4:7, 2 * hp:2 * hp + 2, :], ALU.mult)

                    # attn @ v (+ denominator row), both heads into one
                    # 2-plane PSUM tile so the den copy covers the pair
                    op = opp.tile([65, 2, NQ], F32, tag="op")
                    for j in range(2):
                        hh = 2 * hp + j
                        for c in range(C):
                            nc.tensor.matmul(op[:, j, :],
                                             lhsT=vtp[:, c, 65 * hh:65 * hh + 65],
                                             rhs=texp2d[:, c, j, :],
                                             start=(c == 0), stop=(c == C - 1))
                    h0 = 8 * g + 2 * hp
                    t = h0 // 2
                    for j in range(2):
                        nc.scalar.activation(acc[b][64 * j:64 * j + 64, t, :],
                                             op[0:64, j, :], AF.Copy)
                    nc.vector.tensor_copy(
                        den[b][32 * (h0 // 4):32 * (h0 // 4) + 1,
                               h0 % 4:h0 % 4 + 2, :],
                        op[64:65, :, :])

        # ---- output phase ----
        # constants used only here; queued once attention is underway
        nc.sync.dma_start(shv_sb, shv)
        nc.sync.dma_start(sel, seld)
        nc.sync.dma_start(shp_sb, shp)
        nc.sync.dma_start(wp_sb, wp.rearrange("(c p) n -> p c n", p=128))
        for b in range(BC):
            nc.sync.dma_start(
                den2[b],
                den[b].rearrange("(a c) d e -> a c d e", c=32)[:, 0, :, :])
            rec = den2[b]
            nc.vector.reciprocal(rec, den2[b])
            nc.vector.tensor_copy(recb[b], rec)
            hsw = hswp.tile([128, 8, NQ], BF16, tag="hsw")
            for tp in range(4):
                # two t-tiles per round share a PSUM recip-broadcast tile so
                # the DVE chain runs on [128, 392] slabs
                rep = mmp.tile([128, 512], F32, tag="mm", name="rep")
                for ti in range(2):
                    nc.tensor.matmul(rep[:, 196 * ti:196 * ti + 196],
                                     lhsT=sel[:, 2 * tp + ti, :], rhs=recb[b],
                                     start=True, stop=True)
                # v_t = acc*recip + shift ; hsw6 = v_t * clamp(v_t+3,0,6)
                t1 = tmpp.tile([128, 2, NQ], BF16, tag="t1")
                nc.vector.tensor_tensor(
                    t1, acc[b][:, 2 * tp:2 * tp + 2, :],
                    rep[:, 0:392].rearrange("p (a q) -> p a q", q=NQ),
                    ALU.mult)
                vv = tmpp.tile([128, 2, NQ], BF16, tag="vv")
                for ti in range(2):
                    nc.vector.tensor_scalar_add(
                        vv[:, ti, :], t1[:, ti, :],
                        shv_sb[:, 2 * tp + ti:2 * tp + ti + 1])
                t3 = t1
                nc.vector.tensor_scalar(t3, vv, -3.0, 3.0, ALU.max, ALU.min)
                nc.vector.scalar_tensor_tensor(hsw[:, 2 * tp:2 * tp + 2, :],
                                               t3, 3.0, vv,
                                               ALU.add, ALU.mult)
            for mt, msz in ((0, 128), (1, 68)):
                po = mmp.tile([128, 512], F32, tag="mm", name="po")
                nc.tensor.matmul(po[:msz, :], lhsT=ones1[0:1, 0:msz],
                                 rhs=shp_sb, start=True, stop=False,
                                 skip_group_check=True)
                for kk in range(8):
                    nc.tensor.matmul(
                        po[:msz, :],
                        lhsT=hsw[:, kk, 128 * mt:128 * mt + msz],
                        rhs=wp_sb[:, kk, :], start=False,
                        stop=(kk == 7), skip_group_check=True)
                fin = finp.tile([128, OUT], F32, tag="fin")
                nc.scalar.activation(fin[:msz, :], po[:msz, :], AF.Copy)
                nc.sync.dma_start(out[b, 128 * mt:128 * mt + msz, :],
                                  fin[:msz, :])
    nc.compile()
    return nc


def _prepare_in_maps(inputs):
    inp = {k: np.asarray(v) for k, v in inputs.items()}
    x = inp["x"].astype(np.float32)          # [32, 784, 384]
    Wkv, Wq, Wp = inp["Wkv"], inp["Wq"], inp["Wp"]
    biases, idxs = inp["biases"], inp["idxs"].astype(np.int64)

    s_kv = inp["kv_w"] / np.sqrt(inp["kv_var"] + EPS)
    wkv = (Wkv * s_kv[:, None]).astype(np.float32)
    sh_kv = (inp["kv_b"] - inp["kv_mean"] * s_kv).astype(np.float32)
    wkv3 = wkv.reshape(H, KD + D, IN)
    sh3 = sh_kv.reshape(H, KD + D)
    wkT = np.ascontiguousarray(wkv3[:, :KD, :].reshape(H * KD, IN).T).astype(
        ml_dtypes.bfloat16)
    sh_k = np.ascontiguousarray(sh3[:, :KD].reshape(H * KD))
    wvT = np.ascontiguousarray(wkv3[:, KD:, :].reshape(H * D, IN).T).astype(
        ml_dtypes.bfloat16)
    sh_v = np.ascontiguousarray(sh3[:, KD:].reshape(H * D))

    s_q = inp["q_w"] / np.sqrt(inp["q_var"] + EPS)
    wqT = np.ascontiguousarray((Wq * (s_q * SCALE)[:, None]).T).astype(
        ml_dtypes.bfloat16)
    sh_q = ((inp["q_b"] - inp["q_mean"] * s_q) * SCALE).astype(np.float32)

    s_p = inp["p_w"] / np.sqrt(inp["p_var"] + EPS)
    wpT = np.ascontiguousarray(((Wp * s_p[:, None]) / 6.0).T).astype(
        ml_dtypes.bfloat16)
    sh_p = (inp["p_b"] - inp["p_mean"] * s_p).astype(np.float32)

    eb = np.exp(biases.astype(np.float64))[:, idxs]      # [16, 196, 784]
    eb = eb.transpose(0, 2, 1).reshape(H, C, MC, NQ)
    eb = eb.transpose(2, 0, 1, 3).reshape(MC, G, HG, C, NQ)
    eb = np.ascontiguousarray(eb.transpose(0, 1, 3, 2, 4)).astype(
        ml_dtypes.bfloat16)                              # [MC, G, C, HG, NQ]

    xs = x.reshape(B, RES, RES, IN)[:, ::STRIDE, ::STRIDE].reshape(B, NQ, IN)

    shk_h = np.ascontiguousarray(sh_k.reshape(4, 128).T)
    shq_h = np.ascontiguousarray(sh_q.reshape(4, 128).T)
    shv_h = np.ascontiguousarray(sh_v.reshape(8, 128).T)
    shp_h = np.ascontiguousarray(sh_p.reshape(1, OUT))

    sel_h = np.zeros((16, 8, 128), np.float32)
    for t in range(8):
        sel_h[2 * t, t, 0:64] = 1.0
        sel_h[2 * t + 1, t, 64:128] = 1.0
    sel_h = sel_h.astype(ml_dtypes.bfloat16)
    shared = {"wk": wkT, "wv": wvT, "wq": wqT, "wp": wpT, "shk": shk_h,
              "shq": shq_h, "shv": shv_h, "shp": shp_h, "ebias": eb,
              "seld": sel_h}
    in_maps = []
    for i in range(NCORES):
        xb = x[BC * i:BC * i + BC]
        xsb = xs[BC * i:BC * i + BC]
        m = dict(shared)
        m["xT"] = np.ascontiguousarray(xb.transpose(0, 2, 1)).astype(
            ml_dtypes.bfloat16)
        m["xsT"] = np.ascontiguousarray(xsb.transpose(0, 2, 1)).astype(
            ml_dtypes.bfloat16)
        in_maps.append(m)
    return in_maps


def kernel(**inputs):
    global _NC_CACHE, LAST_RESULTS
    in_maps = _prepare_in_maps(inputs)
    if _NC_CACHE is None:
        _NC_CACHE = _build_nc()
    res = run_bass_kernel_spmd(_NC_CACHE, in_maps,
                               core_ids=list(range(NCORES)), trace=TRACE)
    LAST_RESULTS = res
    return np.concatenate([res.results[i]["out"] for i in range(NCORES)],
                          axis=0)


# revision 44
# speedup vs baseline: 1.0205x; 1.0205x over previous
"""AttentionSubsample Trainium2 kernel: 8-core data-parallel over batch.

Layout strategy (per core, 4 batch elements):
  - All matmuls contract over the SBUF partition dim.
  - kv "k" part + q computed feature-major [feat, tok]; BN folded into weights
    (scale) and eviction bias (shift, per-partition).
  - v computed token-major [tok, feat] so attn@v needs no transpose; its BN
    shift is applied after attention via the softmax-denominator identity.
  - scores computed as s.T [ktok, qtok] per (head, chunk); bias applied as
    exp(s)*exp(bias) with a host-gathered exp-bias table; softmax sum over
    ktok obtained by appending a ones-column to v (row 64 of attn@v output).
  - output projection computed token-major directly (lhsT = hardswish acts),
    BN shift via a K=1 ones-row matmul, so the final store is contiguous.
  - matmul cost is set by the MOVING operand dtype: fp32 = 4 cyc/row, fp32r =
    1 cyc/row only at moving >= 256, bf16 = 1 cyc/row always. So the big
    >=256-moving matmuls (k, v, proj, batched q) keep fp32r moving operands,
    while the 196-moving ones (scores, attn@v, recip-broadcast) use bf16
    moving operands (q_sb, texp, rec) with full-precision fp32r stationaries.
  - exp-bias table is bf16 and loaded once per head-group (not per batch);
    the bias multiply runs on DVE in bf16 (2x mode); v/acc PSUM evictions run
    on the otherwise-idle GPSIMD (Pool) engine.
"""

import sys

sys.path.insert(0, "/opt/trn_rl_repo")

from contextlib import ExitStack

import numpy as np
import ml_dtypes

import concourse.bass as bass
import concourse.tile as tile
from concourse import bacc
from concourse import mybir
from concourse.bass_utils import run_bass_kernel_spmd

F32 = mybir.dt.float32
F32R = mybir.dt.float32r
BF16 = mybir.dt.bfloat16
ALU = mybir.AluOpType
AF = mybir.ActivationFunctionType

B, N, NQ, IN, H, KD, D, OUT = 32, 784, 196, 384, 16, 32, 64, 512
HID, DH = 1536, 1024
RES, RES_, STRIDE = 28, 14, 2
SCALE = KD ** -0.5
EPS = 1e-5
NCORES = 8
BC = B // NCORES          # 4 batch elems per core
C, MC = 7, 112            # key-token chunks: 7 x 112 = 784
G, HG = 2, 8              # 2 head-groups of 8 heads

TRACE = False
LAST_RESULTS = None

_NC_CACHE = None


def _build_nc():
    nc = bacc.Bacc("TRN2", target_bir_lowering=False, debug=False,
                   num_devices=NCORES)

    xT = nc.dram_tensor("xT", [BC, IN, N], BF16, kind="ExternalInput").ap()
    xsT = nc.dram_tensor("xsT", [BC, IN, NQ], BF16, kind="ExternalInput").ap()
    wk = nc.dram_tensor("wk", [IN, 512], BF16, kind="ExternalInput").ap()
    wv = nc.dram_tensor("wv", [IN, DH], BF16, kind="ExternalInput").ap()
    wq = nc.dram_tensor("wq", [IN, 512], BF16, kind="ExternalInput").ap()
    wp = nc.dram_tensor("wp", [DH, OUT], BF16, kind="ExternalInput").ap()
    shk = nc.dram_tensor("shk", [128, 4], F32, kind="ExternalInput").ap()
    shq = nc.dram_tensor("shq", [128, 4], F32, kind="ExternalInput").ap()
    shv = nc.dram_tensor("shv", [128, 8], F32, kind="ExternalInput").ap()
    shp = nc.dram_tensor("shp", [1, OUT], F32R, kind="ExternalInput").ap()
    ebias = nc.dram_tensor("ebias", [MC, G, C, HG, NQ], BF16,
                           kind="ExternalInput").ap()
    seld = nc.dram_tensor("seld", [16, 8, 128], BF16, kind="ExternalInput").ap()
    out = nc.dram_tensor("out", [BC, NQ, OUT], F32, kind="ExternalOutput").ap()

    with tile.TileContext(nc) as tc, ExitStack() as ctx:
        singles = ctx.enter_context(tc.tile_pool(name="singles", bufs=1))
        biasp = ctx.enter_context(tc.tile_pool(name="biasp", bufs=2))
        kqp = ctx.enter_context(tc.tile_pool(name="kqp", bufs=2))
        vp = ctx.enter_context(tc.tile_pool(name="vp", bufs=2))
        texpp = ctx.enter_context(tc.tile_pool(name="texpp", bufs=3))
        accp = ctx.enter_context(tc.tile_pool(name="accp", bufs=1))
        tmpp = ctx.enter_context(tc.tile_pool(name="tmpp", bufs=2))
        hswp = ctx.enter_context(tc.tile_pool(name="hswp", bufs=1))
        finp = ctx.enter_context(tc.tile_pool(name="finp", bufs=2))
        mmp = ctx.enter_context(tc.tile_pool(name="mmp", bufs=2, space="PSUM"))
        scp = ctx.enter_context(tc.tile_pool(name="scp", bufs=2, space="PSUM"))
        opp = ctx.enter_context(tc.tile_pool(name="opp", bufs=2, space="PSUM"))

        # --- persistent SBUF, DMAs ordered by first use (the cost model
        # serializes all DMA traffic, so queue order gates the startup) ---
        # DMAs ordered to match the schedule's start (q matmuls run first):
        # q inputs on the SP queue, k inputs on the Act queue so each path's
        # completion wait is narrow.
        wq_sb = singles.tile([128, 3, 512], BF16)
        nc.sync.dma_start(wq_sb, wq.rearrange("(c p) n -> p c n", p=128))
        shq_sb = singles.tile([128, 4], F32)
        nc.sync.dma_start(shq_sb, shq)
        # subsampled x for all 4 batch elems, loaded once
        xstb = singles.tile([128, 3, BC, NQ], BF16)
        for bb in range(BC):
            nc.sync.dma_start(xstb[:, :, bb, :],
                              xsT[bb].rearrange("(c p) n -> p c n", p=128))
        wk_sb = singles.tile([128, 3, 512], BF16)
        nc.scalar.dma_start(wk_sb, wk.rearrange("(c p) n -> p c n", p=128))
        shk_sb = singles.tile([128, 4], F32)
        nc.scalar.dma_start(shk_sb, shk)
        # x for all 4 batch elems, loaded once and reused by both head groups
        xtb4 = [singles.tile([128, 3, N], BF16, name=f"xtb{b}")
                for b in range(BC)]
        nc.scalar.dma_start(xtb4[0], xT[0].rearrange("(c p) n -> p c n", p=128))
        wv_sb = singles.tile([128, 3, DH], BF16)
        nc.sync.dma_start(wv_sb, wv.rearrange("(c p) n -> p c n", p=128))
        ones1 = singles.tile([1, 128], F32R)
        nc.gpsimd.memset(ones1.bitcast(F32), 1.0)
        # output-phase constants are DMA'd after the attention loop starts;
        # declared here, loaded right before the output phase
        wp_sb = singles.tile([128, 8, OUT], BF16)
        shv_sb = singles.tile([128, 8], F32)
        shp_sb = singles.tile([1, OUT], F32R)
        # sel[:, t, :] is a [16, 128] 0/1 matrix: sel[i, t, m] = 1 iff head i
        # feeds output row m of feature-tile t (rows 0-63 <- head 2t, 64-127
        # <- head 2t+1). Used to broadcast softmax reciprocals across rows.
        sel = singles.tile([16, 8, 128], BF16)

        acc = [accp.tile([128, 8, NQ], BF16, name=f"acc{b}") for b in range(BC)]
        # denominator staging: head h=4g+hh -> partition 32*hh, block g
        den = [accp.tile([128, 4, NQ], F32, name=f"den{b}") for b in range(BC)]
        den2 = [accp.tile([16, NQ], F32, name=f"den2{b}") for b in range(BC)]
        recb = [accp.tile([16, NQ], BF16, name=f"recb{b}") for b in range(BC)]

        for g in range(G):
            # exp-bias table for this head group (reused by all 4 batches),
            # c-major [ktok, chunk, head-in-group, qtok] to match texp2d
            ebias_g = biasp.tile([MC, C, HG, NQ], BF16, tag="ebias")
            nc.sync.dma_start(ebias_g, ebias[:, g, :, :, :])
            if g == 0:
                # remaining x batches: needed from (g0, b1) onwards, queued
                # behind the g0 ebias table
                for bb in range(1, BC):
                    nc.sync.dma_start(xtb4[bb],
                                      xT[bb].rearrange("(c p) n -> p c n",
                                                       p=128))

            # q for this head group, all 4 batches: fp32r-rate bf16 moving
            # 392 (2 b per matmul)
            q_sb = kqp.tile([128, 2, BC, NQ], BF16, tag="q_sb")
            for m2 in range(2):
                for hb in range(2):
                    pq = mmp.tile([128, 512], F32, tag="mm", name="pq")
                    for kk in range(3):
                        nc.tensor.matmul(
                            pq[:, :392],
                            lhsT=wq_sb[:, kk, 256 * g + 128 * m2:
                                       256 * g + 128 * m2 + 128],
                            rhs=xstb[:, kk, 2 * hb:2 * hb + 2, :],
                            start=(kk == 0), stop=(kk == 2))
                    nc.scalar.activation(
                        q_sb[:, m2, 2 * hb:2 * hb + 2, :],
                        pq[:, :392].rearrange("p (b n) -> p b n", n=NQ),
                        AF.Identity,
                        bias=shq_sb[:, 2 * g + m2:2 * g + m2 + 1])

            for b in range(BC):
                xtb = xtb4[b]

                # k for this head group: features [256g, 256g+256), feat-major
                k_sb = kqp.tile([128, 2, N], BF16, tag="k_sb")
                for m2 in range(2):
                    for n2 in range(2):
                        pk = mmp.tile([128, 512], F32, tag="mm", name="pk")
                        for kk in range(3):
                            nc.tensor.matmul(
                                pk[:, :392],
                                lhsT=wk_sb[:, kk, 256 * g + 128 * m2:
                                           256 * g + 128 * m2 + 128],
                                rhs=xtb[:, kk, 392 * n2:392 * n2 + 392],
                                start=(kk == 0), stop=(kk == 2))
                        nc.scalar.activation(
                            k_sb[:, m2, 392 * n2:392 * n2 + 392],
                            pk[:, :392], AF.Identity,
                            bias=shk_sb[:, 2 * g + m2:2 * g + m2 + 1])

                # v token-major for this head group (512 features), with an
                # all-ones column appended per head for the softmax denominator
                vtp = vp.tile([MC, C, 8 * 65], BF16, tag="vtp")
                ones_cols = vtp.rearrange("p c (h e) -> p c h e", e=65)[:, :, :, 64:65]
                nc.vector.memset(ones_cols, 1.0)
                for c in range(C):
                    pv = mmp.tile([128, 512], F32, tag="mm", name="pv")
                    for kk in range(3):
                        nc.tensor.matmul(
                            pv[:MC, :],
                            lhsT=xtb[:, kk, MC * c:MC * c + MC],
                            rhs=wv_sb[:, kk, 512 * g:512 * g + 512],
                            start=(kk == 0), stop=(kk == 2))
                    nc.vector.tensor_copy(
                        vtp.rearrange("p c (h e) -> p c h e", e=65)[:, c, :, 0:64],
                        pv[:MC, :].rearrange("p (h d) -> p h d", d=64))

                # scores + exp + bias-mult, per head pair. texp is c-major
                # [112, chunk, head-of-pair, 196] so each exp output is a
                # contiguous block covering BOTH heads: each score tile packs
                # (chunk, head) as (plane, 196-slot) and one exp drains it.
                for hp in range(HG // 2):
                    texp2d = texpp.tile([MC, C, 2, NQ], BF16, tag="texp")
                    for ct, cs in enumerate(((0, 1), (2, 3), (4, 5), (6,))):
                        # plane j holds head j's chunks (196-slots) so each
                        # PSUM bank sees matmuls from a single PE row band
                        sc = scp.tile([MC, 2, 512], F32, tag="sc",
                                      name=f"sc{ct}")
                        for si, c in enumerate(cs):
                            for j in range(2):
                                hh = 2 * hp + j
                                pb = 32 * (hh % 4)
                                m2 = hh // 4
                                nc.tensor.matmul(
                                    sc[:, j, 196 * si:196 * si + 196],
                                    lhsT=k_sb[pb:pb + 32, m2, MC * c:MC * c + MC],
                                    rhs=q_sb[pb:pb + 32, m2, b, :],
                                    start=True, stop=True,
                                    tile_position=(pb, 0),
                                    skip_group_check=True)
                        if len(cs) == 2:
                            nc.scalar.activation(
                                texp2d[:, cs[0]:cs[0] + 2, :, :],
                                sc[:, :, 0:392].rearrange(
                                    "p a (b q) -> p b a q", q=196), AF.Exp)
                        else:
                            nc.scalar.activation(
                                texp2d[:, 6, :, :],
                                sc[:, :, 0:196], AF.Exp)
                    nc.vector.tensor_tensor(
                        texp2d[:, 0:4, :, :], texp2d[:, 0:4, :, :],
                        ebias_g[:, 0:4, 2 * hp:2 * hp + 2, :], ALU.mult)
                    nc.vector.tensor_tensor(
                        texp2d[:, 4:7, :, :], texp2d[:, 4:7, :, :],
                        ebias_g[:, 4:7, 2 * hp:2 * hp + 2, :], ALU.mult)

                    # attn @ v (+ denominator row), both heads into one
                    # 2-plane PSUM tile so the den copy covers the pair
                    op = opp.tile([65, 2, NQ], F32, tag="op")
                    for j in range(2):
                        hh = 2 * hp + j
                        for c in range(C):
                            nc.tensor.matmul(op[:, j, :],
                                             lhsT=vtp[:, c, 65 * hh:65 * hh + 65],
                                             rhs=texp2d[:, c, j, :],
                                             start=(c == 0), stop=(c == C - 1))
                    h0 = 8 * g + 2 * hp
                    t = h0 // 2
                    for j in range(2):
                        nc.scalar.activation(acc[b][64 * j:64 * j + 64, t, :],
                                             op[0:64, j, :], AF.Copy)
                    nc.vector.tensor_copy(
                        den[b][32 * (h0 // 4):32 * (h0 // 4) + 1,
                               h0 % 4:h0 % 4 + 2, :],
                        op[64:65, :, :])

        # ---- output phase ----
        # constants used only here; queued once attention is underway
        nc.sync.dma_start(shv_sb, shv)
        nc.sync.dma_start(sel, seld)
        nc.sync.dma_start(shp_sb, shp)
        nc.sync.dma_start(wp_sb, wp.rearrange("(c p) n -> p c n", p=128))
        for b in range(BC):
            nc.sync.dma_start(
                den2[b],
                den[b].rearrange("(a c) d e -> a c d e", c=32)[:, 0, :, :])
            rec = den2[b]
            nc.vector.reciprocal(rec, den2[b])
            nc.vector.tensor_copy(recb[b], rec)
            hsw = hswp.tile([128, 8, NQ], BF16, tag="hsw")
            for tp in range(4):
                # two t-tiles per round share a PSUM recip-broadcast tile so
                # the DVE chain runs on [128, 392] slabs
                rep = mmp.tile([128, 512], F32, tag="mm", name="rep")
                for ti in range(2):
                    nc.tensor.matmul(rep[:, 196 * ti:196 * ti + 196],
                                     lhsT=sel[:, 2 * tp + ti, :], rhs=recb[b],
                                     start=True, stop=True)
                # v_t = acc*recip + shift ; hsw6 = v_t * clamp(v_t+3,0,6)
                t1 = tmpp.tile([128, 2, NQ], BF16, tag="t1")
                nc.vector.tensor_tensor(
                    t1, acc[b][:, 2 * tp:2 * tp + 2, :],
                    rep[:, 0:392].rearrange("p (a q) -> p a q", q=NQ),
                    ALU.mult)
                vv = tmpp.tile([128, 2, NQ], BF16, tag="vv")
                for ti in range(2):
                    nc.vector.tensor_scalar_add(
                        vv[:, ti, :], t1[:, ti, :],
                        shv_sb[:, 2 * tp + ti:2 * tp + ti + 1])
                t3 = t1
                nc.vector.tensor_scalar(t3, vv, -3.0, 3.0, ALU.max, ALU.min)
                nc.vector.scalar_tensor_tensor(hsw[:, 2 * tp:2 * tp + 2, :],
                                               t3, 3.0, vv,
                                               ALU.add, ALU.mult)
            for mt, msz in ((0, 128), (1, 68)):
                po = mmp.tile([128, 512], F32, tag="mm", name="po")
                nc.tensor.matmul(po[:msz, :], lhsT=ones1[0:1, 0:msz],
                                 rhs=shp_sb, start=True, stop=False,
                                 skip_group_check=True)
                for kk in range(8):
                    nc.tensor.matmul(
                        po[:msz, :],
                        lhsT=hsw[:, kk, 128 * mt:128 * mt + msz],
                        rhs=wp_sb[:, kk, :], start=False,
                        stop=(kk == 7), skip_group_check=True)
                fin = finp.tile([128, OUT], F32, tag="fin")
                nc.scalar.activation(fin[:msz, :], po[:msz, :], AF.Copy)
                nc.sync.dma_start(out[b, 128 * mt:128 * mt + msz, :],
                                  fin[:msz, :])
    nc.compile()
    return nc


def _prepare_in_maps(inputs):
    inp = {k: np.asarray(v) for k, v in inputs.items()}
    x = inp["x"].astype(np.float32)          # [32, 784, 384]
    Wkv, Wq, Wp = inp["Wkv"], inp["Wq"], inp["Wp"]
    biases, idxs = inp["biases"], inp["idxs"].astype(np.int64)

    s_kv = inp["kv_w"] / np.sqrt(inp["kv_var"] + EPS)
    wkv = (Wkv * s_kv[:, None]).astype(np.float32)
    sh_kv = (inp["kv_b"] - inp["kv_mean"] * s_kv).astype(np.float32)
    wkv3 = wkv.reshape(H, KD + D, IN)
    sh3 = sh_kv.reshape(H, KD + D)
    wkT = np.ascontiguousarray(wkv3[:, :KD, :].reshape(H * KD, IN).T).astype(
        ml_dtypes.bfloat16)
    sh_k = np.ascontiguousarray(sh3[:, :KD].reshape(H * KD))
    wvT = np.ascontiguousarray(wkv3[:, KD:, :].reshape(H * D, IN).T).astype(
        ml_dtypes.bfloat16)
    sh_v = np.ascontiguousarray(sh3[:, KD:].reshape(H * D))

    s_q = inp["q_w"] / np.sqrt(inp["q_var"] + EPS)
    wqT = np.ascontiguousarray((Wq * (s_q * SCALE)[:, None]).T).astype(
        ml_dtypes.bfloat16)
    sh_q = ((inp["q_b"] - inp["q_mean"] * s_q) * SCALE).astype(np.float32)

    s_p = inp["p_w"] / np.sqrt(inp["p_var"] + EPS)
    wpT = np.ascontiguousarray(((Wp * s_p[:, None]) / 6.0).T).astype(
        ml_dtypes.bfloat16)
    sh_p = (inp["p_b"] - inp["p_mean"] * s_p).astype(np.float32)

    eb = np.exp(biases.astype(np.float64))[:, idxs]      # [16, 196, 784]
    eb = eb.transpose(0, 2, 1).reshape(H, C, MC, NQ)
    eb = eb.transpose(2, 0, 1, 3).reshape(MC, G, HG, C, NQ)
    eb = np.ascontiguousarray(eb.transpose(0, 1, 3, 2, 4)).astype(
        ml_dtypes.bfloat16)                              # [MC, G, C, HG, NQ]

    xs = x.reshape(B, RES, RES, IN)[:, ::STRIDE, ::STRIDE].reshape(B, NQ, IN)

    shk_h = np.ascontiguousarray(sh_k.reshape(4, 128).T)
    shq_h = np.ascontiguousarray(sh_q.reshape(4, 128).T)
    shv_h = np.ascontiguousarray(sh_v.reshape(8, 128).T)
    shp_h = np.ascontiguousarray(sh_p.reshape(1, OUT))

    sel_h = np.zeros((16, 8, 128), np.float32)
    for t in range(8):
        sel_h[2 * t, t, 0:64] = 1.0
        sel_h[2 * t + 1, t, 64:128] = 1.0
    sel_h = sel_h.astype(ml_dtypes.bfloat16)
    shared = {"wk": wkT, "wv": wvT, "wq": wqT, "wp": wpT, "shk": shk_h,
              "shq": shq_h, "shv": shv_h, "shp": shp_h, "ebias": eb,
              "seld": sel_h}
    in_maps = []
    for i in range(NCORES):
        xb = x[BC * i:BC * i + BC]
        xsb = xs[BC * i:BC * i + BC]
        m = dict(shared)
        m["xT"] = np.ascontiguousarray(xb.transpose(0, 2, 1)).astype(
            ml_dtypes.bfloat16)
        m["xsT"] = np.ascontiguousarray(xsb.transpose(0, 2, 1)).astype(
            ml_dtypes.bfloat16)
        in_maps.append(m)
    return in_maps


def kernel(**inputs):
    global _NC_CACHE, LAST_RESULTS
    in_maps = _prepare_in_maps(inputs)
    if _NC_CACHE is None:
        _NC_CACHE = _build_nc()
    res = run_bass_kernel_spmd(_NC_CACHE, in_maps,
                               core_ids=list(range(NCORES)), trace=TRACE)
    LAST_RESULTS = res
    return np.concatenate([res.results[i]["out"] for i in range(NCORES)],
                          axis=0)


# revision 45
# speedup vs baseline: 1.1135x; 1.0911x over previous
"""AttentionSubsample Trainium2 kernel: 8-core data-parallel over batch.

Layout strategy (per core, 4 batch elements):
  - All matmuls contract over the SBUF partition dim.
  - kv "k" part + q computed feature-major [feat, tok]; BN folded into weights
    (scale) and eviction bias (shift, per-partition).
  - v computed token-major [tok, feat] so attn@v needs no transpose; its BN
    shift is applied after attention via the softmax-denominator identity.
  - scores computed as s.T [ktok, qtok] per (head, chunk); bias applied as
    exp(s)*exp(bias) with a host-gathered exp-bias table; softmax sum over
    ktok obtained by appending a ones-column to v (row 64 of attn@v output).
  - output projection computed token-major directly (lhsT = hardswish acts),
    BN shift via a K=1 ones-row matmul, so the final store is contiguous.
  - matmul cost is set by the MOVING operand dtype: fp32 = 4 cyc/row, fp32r =
    1 cyc/row only at moving >= 256, bf16 = 1 cyc/row always. So the big
    >=256-moving matmuls (k, v, proj, batched q) keep fp32r moving operands,
    while the 196-moving ones (scores, attn@v, recip-broadcast) use bf16
    moving operands (q_sb, texp, rec) with full-precision fp32r stationaries.
  - exp-bias table is bf16 and loaded once per head-group (not per batch);
    the bias multiply runs on DVE in bf16 (2x mode); v/acc PSUM evictions run
    on the otherwise-idle GPSIMD (Pool) engine.
"""

import sys

sys.path.insert(0, "/opt/trn_rl_repo")

from contextlib import ExitStack

import numpy as np
import ml_dtypes

import concourse.bass as bass
import concourse.tile as tile
from concourse import bacc
from concourse import mybir
from concourse.bass_utils import run_bass_kernel_spmd

F32 = mybir.dt.float32
F32R = mybir.dt.float32r
BF16 = mybir.dt.bfloat16
ALU = mybir.AluOpType
AF = mybir.ActivationFunctionType

B, N, NQ, IN, H, KD, D, OUT = 32, 784, 196, 384, 16, 32, 64, 512
HID, DH = 1536, 1024
RES, RES_, STRIDE = 28, 14, 2
SCALE = KD ** -0.5
EPS = 1e-5
NCORES = 8
BC = B // NCORES          # 4 batch elems per core
C, MC = 7, 112            # key-token chunks: 7 x 112 = 784
G, HG = 2, 8              # 2 head-groups of 8 heads

TRACE = False
LAST_RESULTS = None

_NC_CACHE = None


def _build_nc():
    nc = bacc.Bacc("TRN2", target_bir_lowering=False, debug=False,
                   num_devices=NCORES)

    xT = nc.dram_tensor("xT", [BC, IN, N], BF16, kind="ExternalInput").ap()
    xsT = nc.dram_tensor("xsT", [BC, IN, NQ], BF16, kind="ExternalInput").ap()
    wk = nc.dram_tensor("wk", [IN, 512], BF16, kind="ExternalInput").ap()
    wv = nc.dram_tensor("wv", [IN, DH], BF16, kind="ExternalInput").ap()
    wq = nc.dram_tensor("wq", [IN, 512], BF16, kind="ExternalInput").ap()
    wp = nc.dram_tensor("wp", [DH, OUT], BF16, kind="ExternalInput").ap()
    shk = nc.dram_tensor("shk", [128, 4], F32, kind="ExternalInput").ap()
    shq = nc.dram_tensor("shq", [128, 4], F32, kind="ExternalInput").ap()
    shv = nc.dram_tensor("shv", [128, 8], F32, kind="ExternalInput").ap()
    shp = nc.dram_tensor("shp", [1, OUT], F32R, kind="ExternalInput").ap()
    ebias = nc.dram_tensor("ebias", [MC, G, C, HG, NQ], BF16,
                           kind="ExternalInput").ap()
    seld = nc.dram_tensor("seld", [16, 8, 128], BF16, kind="ExternalInput").ap()
    out = nc.dram_tensor("out", [BC, NQ, OUT], F32, kind="ExternalOutput").ap()

    with tile.TileContext(nc) as tc, ExitStack() as ctx:
        singles = ctx.enter_context(tc.tile_pool(name="singles", bufs=1))
        biasp = ctx.enter_context(tc.tile_pool(name="biasp", bufs=2))
        kqp = ctx.enter_context(tc.tile_pool(name="kqp", bufs=2))
        vp = ctx.enter_context(tc.tile_pool(name="vp", bufs=2))
        texpp = ctx.enter_context(tc.tile_pool(name="texpp", bufs=3))
        accp = ctx.enter_context(tc.tile_pool(name="accp", bufs=1))
        tmpp = ctx.enter_context(tc.tile_pool(name="tmpp", bufs=2))
        hswp = ctx.enter_context(tc.tile_pool(name="hswp", bufs=1))
        finp = ctx.enter_context(tc.tile_pool(name="finp", bufs=2))
        mmp = ctx.enter_context(tc.tile_pool(name="mmp", bufs=2, space="PSUM"))
        scp = ctx.enter_context(tc.tile_pool(name="scp", bufs=2, space="PSUM"))
        opp = ctx.enter_context(tc.tile_pool(name="opp", bufs=2, space="PSUM"))

        # --- persistent SBUF, DMAs ordered by first use (the cost model
        # serializes all DMA traffic, so queue order gates the startup) ---
        # DMAs ordered to match the schedule's start (q matmuls run first):
        # q inputs on the SP queue, k inputs on the Act queue so each path's
        # completion wait is narrow.
        wq_sb = singles.tile([128, 3, 512], BF16)
        nc.sync.dma_start(wq_sb, wq.rearrange("(c p) n -> p c n", p=128))
        shq_sb = singles.tile([128, 4], F32)
        nc.sync.dma_start(shq_sb, shq)
        # subsampled x for all 4 batch elems, loaded once
        xstb = singles.tile([128, 3, BC, NQ], BF16)
        for bb in range(BC):
            nc.sync.dma_start(xstb[:, :, bb, :],
                              xsT[bb].rearrange("(c p) n -> p c n", p=128))
        wk_sb = singles.tile([128, 3, 512], BF16)
        nc.scalar.dma_start(wk_sb, wk.rearrange("(c p) n -> p c n", p=128))
        shk_sb = singles.tile([128, 4], F32)
        nc.scalar.dma_start(shk_sb, shk)
        # x for all 4 batch elems, loaded once and reused by both head groups
        xtb4 = [singles.tile([128, 3, N], BF16, name=f"xtb{b}")
                for b in range(BC)]
        nc.scalar.dma_start(xtb4[0], xT[0].rearrange("(c p) n -> p c n", p=128))
        wv_sb = singles.tile([128, 3, DH], BF16)
        nc.sync.dma_start(wv_sb, wv.rearrange("(c p) n -> p c n", p=128))
        ones1 = singles.tile([1, 128], F32R)
        nc.gpsimd.memset(ones1.bitcast(F32), 1.0)
        # output-phase constants are DMA'd after the attention loop starts;
        # declared here, loaded right before the output phase
        wp_sb = singles.tile([128, 8, OUT], BF16)
        shv_sb = singles.tile([128, 8], F32)
        shp_sb = singles.tile([1, OUT], F32R)
        # sel[:, t, :] is a [16, 128] 0/1 matrix: sel[i, t, m] = 1 iff head i
        # feeds output row m of feature-tile t (rows 0-63 <- head 2t, 64-127
        # <- head 2t+1). Used to broadcast softmax reciprocals across rows.
        sel = singles.tile([16, 8, 128], BF16)

        acc = [accp.tile([128, 8, NQ], BF16, name=f"acc{b}") for b in range(BC)]
        # denominator staging: head h=4g+hh -> partition 32*hh, block g
        den = [accp.tile([128, 4, NQ], F32, name=f"den{b}") for b in range(BC)]
        den2 = [accp.tile([16, NQ], F32, name=f"den2{b}") for b in range(BC)]
        recb = [accp.tile([16, NQ], BF16, name=f"recb{b}") for b in range(BC)]

        for g in range(G):
            # exp-bias table for this head group (reused by all 4 batches),
            # c-major [ktok, chunk, head-in-group, qtok] to match texp2d
            ebias_g = biasp.tile([MC, C, HG, NQ], BF16, tag="ebias")
            nc.sync.dma_start(ebias_g, ebias[:, g, :, :, :])
            if g == 0:
                # remaining x batches: needed from (g0, b1) onwards, queued
                # behind the g0 ebias table
                for bb in range(1, BC):
                    nc.sync.dma_start(xtb4[bb],
                                      xT[bb].rearrange("(c p) n -> p c n",
                                                       p=128))

            # q for this head group, all 4 batches: fp32r-rate bf16 moving
            # 392 (2 b per matmul)
            q_sb = kqp.tile([128, 2, BC, NQ], BF16, tag="q_sb")
            for m2 in range(2):
                for hb in range(2):
                    pq = mmp.tile([128, 512], F32, tag="mm", name="pq")
                    for kk in range(3):
                        nc.tensor.matmul(
                            pq[:, :392],
                            lhsT=wq_sb[:, kk, 256 * g + 128 * m2:
                                       256 * g + 128 * m2 + 128],
                            rhs=xstb[:, kk, 2 * hb:2 * hb + 2, :],
                            start=(kk == 0), stop=(kk == 2))
                    nc.scalar.activation(
                        q_sb[:, m2, 2 * hb:2 * hb + 2, :],
                        pq[:, :392].rearrange("p (b n) -> p b n", n=NQ),
                        AF.Identity,
                        bias=shq_sb[:, 2 * g + m2:2 * g + m2 + 1])

            for b in range(BC):
                xtb = xtb4[b]

                # k for this head group: features [256g, 256g+256), feat-major
                k_sb = kqp.tile([128, 2, N], BF16, tag="k_sb")
                for m2 in range(2):
                    for n2 in range(2):
                        pk = mmp.tile([128, 512], F32, tag="mm", name="pk")
                        for kk in range(3):
                            nc.tensor.matmul(
                                pk[:, :392],
                                lhsT=wk_sb[:, kk, 256 * g + 128 * m2:
                                           256 * g + 128 * m2 + 128],
                                rhs=xtb[:, kk, 392 * n2:392 * n2 + 392],
                                start=(kk == 0), stop=(kk == 2))
                        nc.vector.tensor_scalar_add(
                            k_sb[:, m2, 392 * n2:392 * n2 + 392],
                            pk[:, :392],
                            shk_sb[:, 2 * g + m2:2 * g + m2 + 1])

                # v token-major for this head group (512 features), with an
                # all-ones column appended per head for the softmax denominator
                vtp = vp.tile([MC, C, 8 * 65], BF16, tag="vtp")
                ones_cols = vtp.rearrange("p c (h e) -> p c h e", e=65)[:, :, :, 64:65]
                nc.vector.memset(ones_cols, 1.0)
                for c in range(C):
                    pv = mmp.tile([128, 512], F32, tag="mm", name="pv")
                    for kk in range(3):
                        nc.tensor.matmul(
                            pv[:MC, :],
                            lhsT=xtb[:, kk, MC * c:MC * c + MC],
                            rhs=wv_sb[:, kk, 512 * g:512 * g + 512],
                            start=(kk == 0), stop=(kk == 2))
                    nc.vector.tensor_copy(
                        vtp.rearrange("p c (h e) -> p c h e", e=65)[:, c, :, 0:64],
                        pv[:MC, :].rearrange("p (h d) -> p h d", d=64))

                # scores + exp + bias-mult, per head pair. texp is c-major
                # [112, chunk, head-of-pair, 196] so each exp output is a
                # contiguous block covering BOTH heads: each score tile packs
                # (chunk, head) as (plane, 196-slot) and one exp drains it.
                for hp in range(HG // 2):
                    texp2d = texpp.tile([MC, C, 2, NQ], BF16, tag="texp")
                    for ct, cs in enumerate(((0, 1), (2, 3), (4, 5), (6,))):
                        # plane j holds head j's chunks (196-slots) so each
                        # PSUM bank sees matmuls from a single PE row band
                        sc = scp.tile([MC, 2, 512], F32, tag="sc",
                                      name=f"sc{ct}")
                        for si, c in enumerate(cs):
                            for j in range(2):
                                hh = 2 * hp + j
                                pb = 32 * (hh % 4)
                                m2 = hh // 4
                                nc.tensor.matmul(
                                    sc[:, j, 196 * si:196 * si + 196],
                                    lhsT=k_sb[pb:pb + 32, m2, MC * c:MC * c + MC],
                                    rhs=q_sb[pb:pb + 32, m2, b, :],
                                    start=True, stop=True,
                                    tile_position=(pb, 0),
                                    skip_group_check=True)
                        if len(cs) == 2:
                            nc.scalar.activation(
                                texp2d[:, cs[0]:cs[0] + 2, :, :],
                                sc[:, :, 0:392].rearrange(
                                    "p a (b q) -> p b a q", q=196), AF.Exp)
                        else:
                            nc.scalar.activation(
                                texp2d[:, 6, :, :],
                                sc[:, :, 0:196], AF.Exp)
                    nc.vector.tensor_tensor(
                        texp2d[:, 0:4, :, :], texp2d[:, 0:4, :, :],
                        ebias_g[:, 0:4, 2 * hp:2 * hp + 2, :], ALU.mult)
                    nc.vector.tensor_tensor(
                        texp2d[:, 4:7, :, :], texp2d[:, 4:7, :, :],
                        ebias_g[:, 4:7, 2 * hp:2 * hp + 2, :], ALU.mult)

                    # attn @ v (+ denominator row), both heads into one
                    # 2-plane PSUM tile so the den copy covers the pair
                    op = opp.tile([65, 2, NQ], F32, tag="op")
                    for j in range(2):
                        hh = 2 * hp + j
                        for c in range(C):
                            nc.tensor.matmul(op[:, j, :],
                                             lhsT=vtp[:, c, 65 * hh:65 * hh + 65],
                                             rhs=texp2d[:, c, j, :],
                                             start=(c == 0), stop=(c == C - 1))
                    h0 = 8 * g + 2 * hp
                    t = h0 // 2
                    for j in range(2):
                        nc.scalar.activation(acc[b][64 * j:64 * j + 64, t, :],
                                             op[0:64, j, :], AF.Copy)
                    nc.vector.tensor_copy(
                        den[b][32 * (h0 // 4):32 * (h0 // 4) + 1,
                               h0 % 4:h0 % 4 + 2, :],
                        op[64:65, :, :])

        # ---- output phase ----
        # constants used only here; queued once attention is underway
        nc.sync.dma_start(shv_sb, shv)
        nc.sync.dma_start(sel, seld)
        nc.sync.dma_start(shp_sb, shp)
        nc.sync.dma_start(wp_sb, wp.rearrange("(c p) n -> p c n", p=128))
        for b in range(BC):
            nc.sync.dma_start(
                den2[b],
                den[b].rearrange("(a c) d e -> a c d e", c=32)[:, 0, :, :])
            rec = den2[b]
            nc.vector.reciprocal(rec, den2[b])
            nc.vector.tensor_copy(recb[b], rec)
            hsw = hswp.tile([128, 8, NQ], BF16, tag="hsw")
            for tp in range(4):
                # two t-tiles per round share a PSUM recip-broadcast tile so
                # the DVE chain runs on [128, 392] slabs
                rep = mmp.tile([128, 512], F32, tag="mm", name="rep")
                for ti in range(2):
                    nc.tensor.matmul(rep[:, 196 * ti:196 * ti + 196],
                                     lhsT=sel[:, 2 * tp + ti, :], rhs=recb[b],
                                     start=True, stop=True)
                # v_t = acc*recip + shift ; hsw6 = v_t * clamp(v_t+3,0,6)
                t1 = tmpp.tile([128, 2, NQ], BF16, tag="t1")
                nc.vector.tensor_tensor(
                    t1, acc[b][:, 2 * tp:2 * tp + 2, :],
                    rep[:, 0:392].rearrange("p (a q) -> p a q", q=NQ),
                    ALU.mult)
                vv = tmpp.tile([128, 2, NQ], BF16, tag="vv")
                for ti in range(2):
                    nc.vector.tensor_scalar_add(
                        vv[:, ti, :], t1[:, ti, :],
                        shv_sb[:, 2 * tp + ti:2 * tp + ti + 1])
                t3 = t1
                nc.vector.tensor_scalar(t3, vv, -3.0, 3.0, ALU.max, ALU.min)
                nc.vector.scalar_tensor_tensor(hsw[:, 2 * tp:2 * tp + 2, :],
                                               t3, 3.0, vv,
                                               ALU.add, ALU.mult)
            for mt, msz in ((0, 128), (1, 68)):
                po = mmp.tile([128, 512], F32, tag="mm", name="po")
                nc.tensor.matmul(po[:msz, :], lhsT=ones1[0:1, 0:msz],
                                 rhs=shp_sb, start=True, stop=False,
                                 skip_group_check=True)
                for kk in range(8):
                    nc.tensor.matmul(
                        po[:msz, :],
                        lhsT=hsw[:, kk, 128 * mt:128 * mt + msz],
                        rhs=wp_sb[:, kk, :], start=False,
                        stop=(kk == 7), skip_group_check=True)
                fin = finp.tile([128, OUT], F32, tag="fin")
                nc.scalar.activation(fin[:msz, :], po[:msz, :], AF.Copy)
                nc.sync.dma_start(out[b, 128 * mt:128 * mt + msz, :],
                                  fin[:msz, :])
    nc.compile()
    return nc


def _prepare_in_maps(inputs):
    inp = {k: np.asarray(v) for k, v in inputs.items()}
    x = inp["x"].astype(np.float32)          # [32, 784, 384]
    Wkv, Wq, Wp = inp["Wkv"], inp["Wq"], inp["Wp"]
    biases, idxs = inp["biases"], inp["idxs"].astype(np.int64)

    s_kv = inp["kv_w"] / np.sqrt(inp["kv_var"] + EPS)
    wkv = (Wkv * s_kv[:, None]).astype(np.float32)
    sh_kv = (inp["kv_b"] - inp["kv_mean"] * s_kv).astype(np.float32)
    wkv3 = wkv.reshape(H, KD + D, IN)
    sh3 = sh_kv.reshape(H, KD + D)
    wkT = np.ascontiguousarray(wkv3[:, :KD, :].reshape(H * KD, IN).T).astype(
        ml_dtypes.bfloat16)
    sh_k = np.ascontiguousarray(sh3[:, :KD].reshape(H * KD))
    wvT = np.ascontiguousarray(wkv3[:, KD:, :].reshape(H * D, IN).T).astype(
        ml_dtypes.bfloat16)
    sh_v = np.ascontiguousarray(sh3[:, KD:].reshape(H * D))

    s_q = inp["q_w"] / np.sqrt(inp["q_var"] + EPS)
    wqT = np.ascontiguousarray((Wq * (s_q * SCALE)[:, None]).T).astype(
        ml_dtypes.bfloat16)
    sh_q = ((inp["q_b"] - inp["q_mean"] * s_q) * SCALE).astype(np.float32)

    s_p = inp["p_w"] / np.sqrt(inp["p_var"] + EPS)
    wpT = np.ascontiguousarray(((Wp * s_p[:, None]) / 6.0).T).astype(
        ml_dtypes.bfloat16)
    sh_p = (inp["p_b"] - inp["p_mean"] * s_p).astype(np.float32)

    eb = np.exp(biases.astype(np.float64))[:, idxs]      # [16, 196, 784]
    eb = eb.transpose(0, 2, 1).reshape(H, C, MC, NQ)
    eb = eb.transpose(2, 0, 1, 3).reshape(MC, G, HG, C, NQ)
    eb = np.ascontiguousarray(eb.transpose(0, 1, 3, 2, 4)).astype(
        ml_dtypes.bfloat16)                              # [MC, G, C, HG, NQ]

    xs = x.reshape(B, RES, RES, IN)[:, ::STRIDE, ::STRIDE].reshape(B, NQ, IN)

    shk_h = np.ascontiguousarray(sh_k.reshape(4, 128).T)
    shq_h = np.ascontiguousarray(sh_q.reshape(4, 128).T)
    shv_h = np.ascontiguousarray(sh_v.reshape(8, 128).T)
    shp_h = np.ascontiguousarray(sh_p.reshape(1, OUT))

    sel_h = np.zeros((16, 8, 128), np.float32)
    for t in range(8):
        sel_h[2 * t, t, 0:64] = 1.0
        sel_h[2 * t + 1, t, 64:128] = 1.0
    sel_h = sel_h.astype(ml_dtypes.bfloat16)
    shared = {"wk": wkT, "wv": wvT, "wq": wqT, "wp": wpT, "shk": shk_h,
              "shq": shq_h, "shv": shv_h, "shp": shp_h, "ebias": eb,
              "seld": sel_h}
    in_maps = []
    for i in range(NCORES):
        xb = x[BC * i:BC * i + BC]
        xsb = xs[BC * i:BC * i + BC]
        m = dict(shared)
        m["xT"] = np.ascontiguousarray(xb.transpose(0, 2, 1)).astype(
            ml_dtypes.bfloat16)
        m["xsT"] = np.ascontiguousarray(xsb.transpose(0, 2, 1)).astype(
            ml_dtypes.bfloat16)
        in_maps.append(m)
    return in_maps


def kernel(**inputs):
    global _NC_CACHE, LAST_RESULTS
    in_maps = _prepare_in_maps(inputs)
    if _NC_CACHE is None:
        _NC_CACHE = _build_nc()
    res = run_bass_kernel_spmd(_NC_CACHE, in_maps,
                               core_ids=list(range(NCORES)), trace=TRACE)
    LAST_RESULTS = res
    return np.concatenate([res.results[i]["out"] for i in range(NCORES)],
                          axis=0)


# revision 51
# speedup vs baseline: 1.1185x; 1.0045x over previous
"""AttentionSubsample Trainium2 kernel: 8-core data-parallel over batch.

Layout strategy (per core, 4 batch elements):
  - All matmuls contract over the SBUF partition dim.
  - kv "k" part + q computed feature-major [feat, tok]; BN folded into weights
    (scale) and eviction bias (shift, per-partition).
  - v computed token-major [tok, feat] so attn@v needs no transpose; its BN
    shift is applied after attention via the softmax-denominator identity.
  - scores computed as s.T [ktok, qtok] per (head, chunk); bias applied as
    exp(s)*exp(bias) with a host-gathered exp-bias table; softmax sum over
    ktok obtained by appending a ones-column to v (row 64 of attn@v output).
  - output projection computed token-major directly (lhsT = hardswish acts),
    BN shift via a K=1 ones-row matmul, so the final store is contiguous.
  - matmul cost is set by the MOVING operand dtype: fp32 = 4 cyc/row, fp32r =
    1 cyc/row only at moving >= 256, bf16 = 1 cyc/row always. So the big
    >=256-moving matmuls (k, v, proj, batched q) keep fp32r moving operands,
    while the 196-moving ones (scores, attn@v, recip-broadcast) use bf16
    moving operands (q_sb, texp, rec) with full-precision fp32r stationaries.
  - exp-bias table is bf16 and loaded once per head-group (not per batch);
    the bias multiply runs on DVE in bf16 (2x mode); v/acc PSUM evictions run
    on the otherwise-idle GPSIMD (Pool) engine.
"""

import sys

sys.path.insert(0, "/opt/trn_rl_repo")

from contextlib import ExitStack

import numpy as np
import ml_dtypes

import concourse.bass as bass
import concourse.tile as tile
from concourse import bacc
from concourse import mybir
from concourse.bass_utils import run_bass_kernel_spmd

F32 = mybir.dt.float32
F32R = mybir.dt.float32r
BF16 = mybir.dt.bfloat16
ALU = mybir.AluOpType
AF = mybir.ActivationFunctionType

B, N, NQ, IN, H, KD, D, OUT = 32, 784, 196, 384, 16, 32, 64, 512
HID, DH = 1536, 1024
RES, RES_, STRIDE = 28, 14, 2
SCALE = KD ** -0.5
EPS = 1e-5
NCORES = 8
BC = B // NCORES          # 4 batch elems per core
C, MC = 7, 112            # key-token chunks: 7 x 112 = 784
G, HG = 2, 8              # 2 head-groups of 8 heads

TRACE = False
LAST_RESULTS = None

_NC_CACHE = None


def _build_nc():
    nc = bacc.Bacc("TRN2", target_bir_lowering=False, debug=False,
                   num_devices=NCORES)

    xT = nc.dram_tensor("xT", [BC, IN, N], BF16, kind="ExternalInput").ap()
    xsT = nc.dram_tensor("xsT", [IN, BC, NQ], BF16, kind="ExternalInput").ap()
    wk = nc.dram_tensor("wk", [IN, 512], BF16, kind="ExternalInput").ap()
    wv = nc.dram_tensor("wv", [IN, DH], BF16, kind="ExternalInput").ap()
    wq = nc.dram_tensor("wq", [IN, 512], BF16, kind="ExternalInput").ap()
    wp = nc.dram_tensor("wp", [DH, OUT], BF16, kind="ExternalInput").ap()
    shk = nc.dram_tensor("shk", [128, 4], F32, kind="ExternalInput").ap()
    shq = nc.dram_tensor("shq", [128, 4], F32, kind="ExternalInput").ap()
    shv = nc.dram_tensor("shv", [128, 8], F32, kind="ExternalInput").ap()
    shp = nc.dram_tensor("shp", [1, OUT], F32R, kind="ExternalInput").ap()
    ebias = nc.dram_tensor("ebias", [MC, G, C, HG, NQ], BF16,
                           kind="ExternalInput").ap()
    seld = nc.dram_tensor("seld", [16, 8, 128], BF16, kind="ExternalInput").ap()
    out = nc.dram_tensor("out", [BC, NQ, OUT], F32, kind="ExternalOutput").ap()

    with tile.TileContext(nc) as tc, ExitStack() as ctx:
        singles = ctx.enter_context(tc.tile_pool(name="singles", bufs=1))
        biasp = ctx.enter_context(tc.tile_pool(name="biasp", bufs=2))
        kqp = ctx.enter_context(tc.tile_pool(name="kqp", bufs=2))
        vp = ctx.enter_context(tc.tile_pool(name="vp", bufs=2))
        texpp = ctx.enter_context(tc.tile_pool(name="texpp", bufs=3))
        accp = ctx.enter_context(tc.tile_pool(name="accp", bufs=1))
        tmpp = ctx.enter_context(tc.tile_pool(name="tmpp", bufs=2))
        hswp = ctx.enter_context(tc.tile_pool(name="hswp", bufs=1))
        finp = ctx.enter_context(tc.tile_pool(name="finp", bufs=2))
        mmp = ctx.enter_context(tc.tile_pool(name="mmp", bufs=2, space="PSUM"))
        scp = ctx.enter_context(tc.tile_pool(name="scp", bufs=2, space="PSUM"))
        opp = ctx.enter_context(tc.tile_pool(name="opp", bufs=2, space="PSUM"))

        # --- persistent SBUF, DMAs ordered by first use (the cost model
        # serializes all DMA traffic, so queue order gates the startup) ---
        # DMAs ordered to match the schedule's start (q matmuls run first):
        # q inputs on the SP queue, k inputs on the Act queue so each path's
        # completion wait is narrow.
        wq_sb = singles.tile([128, 3, 512], BF16)
        nc.sync.dma_start(wq_sb, wq.rearrange("(c p) n -> p c n", p=128))
        shq_sb = singles.tile([128, 4], F32)
        nc.sync.dma_start(shq_sb, shq)
        # subsampled x for all 4 batch elems, one DMA (host stores [IN, B*NQ])
        xstb = singles.tile([128, 3, BC, NQ], BF16)
        nc.sync.dma_start(
            xstb.rearrange("p c b n -> p c (b n)"),
            xsT.rearrange("(c p) b n -> p c (b n)", p=128))
        wk_sb = singles.tile([128, 3, 512], BF16)
        nc.scalar.dma_start(wk_sb, wk.rearrange("(c p) n -> p c n", p=128))
        shk_sb = singles.tile([128, 4], F32)
        nc.scalar.dma_start(shk_sb, shk)
        # x for all 4 batch elems, loaded once and reused by both head groups
        xtb4 = [singles.tile([128, 3, N], BF16, name=f"xtb{b}")
                for b in range(BC)]
        nc.scalar.dma_start(xtb4[0], xT[0].rearrange("(c p) n -> p c n", p=128))
        wv_sb = singles.tile([128, 3, DH], BF16)
        nc.sync.dma_start(wv_sb, wv.rearrange("(c p) n -> p c n", p=128))
        ones1 = singles.tile([1, 128], F32R)
        nc.gpsimd.memset(ones1.bitcast(F32), 1.0)
        # output-phase constants are DMA'd after the attention loop starts;
        # declared here, loaded right before the output phase
        wp_sb = singles.tile([128, 8, OUT], BF16)
        shv_sb = singles.tile([128, 8], F32)
        shp_sb = singles.tile([1, OUT], F32R)
        # sel[:, t, :] is a [16, 128] 0/1 matrix: sel[i, t, m] = 1 iff head i
        # feeds output row m of feature-tile t (rows 0-63 <- head 2t, 64-127
        # <- head 2t+1). Used to broadcast softmax reciprocals across rows.
        sel = singles.tile([16, 8, 128], BF16)

        acc = [accp.tile([128, 8, NQ], BF16, name=f"acc{b}") for b in range(BC)]
        # denominator staging: head h=4g+hh -> partition 32*hh, block g
        den = [accp.tile([128, 4, NQ], F32, name=f"den{b}") for b in range(BC)]
        den2 = [accp.tile([16, NQ], F32, name=f"den2{b}") for b in range(BC)]
        recb = [accp.tile([16, NQ], BF16, name=f"recb{b}") for b in range(BC)]

        for g in range(G):
            # exp-bias table for this head group (reused by all 4 batches),
            # c-major [ktok, chunk, head-in-group, qtok] to match texp2d
            ebias_g = biasp.tile([MC, C, HG, NQ], BF16, tag="ebias")
            nc.sync.dma_start(ebias_g, ebias[:, g, :, :, :])
            if g == 0:
                # remaining x batches: needed from (g0, b1) onwards, queued
                # behind the g0 ebias table
                for bb in range(1, BC):
                    nc.sync.dma_start(xtb4[bb],
                                      xT[bb].rearrange("(c p) n -> p c n",
                                                       p=128))

            # q for this head group, all 4 batches: fp32r-rate bf16 moving
            # 392 (2 b per matmul)
            q_sb = kqp.tile([128, 2, BC, NQ], BF16, tag="q_sb")
            for m2 in range(2):
                for hb in range(2):
                    pq = mmp.tile([128, 512], F32, tag="mm", name="pq")
                    for kk in range(3):
                        nc.tensor.matmul(
                            pq[:, :392],
                            lhsT=wq_sb[:, kk, 256 * g + 128 * m2:
                                       256 * g + 128 * m2 + 128],
                            rhs=xstb[:, kk, 2 * hb:2 * hb + 2, :],
                            start=(kk == 0), stop=(kk == 2))
                    nc.scalar.activation(
                        q_sb[:, m2, 2 * hb:2 * hb + 2, :],
                        pq[:, :392].rearrange("p (b n) -> p b n", n=NQ),
                        AF.Identity,
                        bias=shq_sb[:, 2 * g + m2:2 * g + m2 + 1])

            for b in range(BC):
                xtb = xtb4[b]

                # k for this head group: features [256g, 256g+256), feat-major
                k_sb = kqp.tile([128, 2, N], BF16, tag="k_sb")
                for m2 in range(2):
                    for n2 in range(2):
                        pk = mmp.tile([128, 512], F32, tag="mm", name="pk")
                        for kk in range(3):
                            nc.tensor.matmul(
                                pk[:, :392],
                                lhsT=wk_sb[:, kk, 256 * g + 128 * m2:
                                           256 * g + 128 * m2 + 128],
                                rhs=xtb[:, kk, 392 * n2:392 * n2 + 392],
                                start=(kk == 0), stop=(kk == 2))
                        nc.vector.tensor_scalar_add(
                            k_sb[:, m2, 392 * n2:392 * n2 + 392],
                            pk[:, :392],
                            shk_sb[:, 2 * g + m2:2 * g + m2 + 1])

                # v token-major for this head group (512 features), with an
                # all-ones column appended per head for the softmax denominator
                vtp = vp.tile([MC, C, 8 * 65], BF16, tag="vtp")
                ones_cols = vtp.rearrange("p c (h e) -> p c h e", e=65)[:, :, :, 64:65]
                nc.vector.memset(ones_cols, 1.0)
                for c in range(C):
                    pv = mmp.tile([128, 512], F32, tag="mm", name="pv")
                    for kk in range(3):
                        nc.tensor.matmul(
                            pv[:MC, :],
                            lhsT=xtb[:, kk, MC * c:MC * c + MC],
                            rhs=wv_sb[:, kk, 512 * g:512 * g + 512],
                            start=(kk == 0), stop=(kk == 2))
                    nc.vector.tensor_copy(
                        vtp.rearrange("p c (h e) -> p c h e", e=65)[:, c, :, 0:64],
                        pv[:MC, :].rearrange("p (h d) -> p h d", d=64))

                # scores + exp + bias-mult, per head pair. texp is c-major
                # [112, chunk, head-of-pair, 196] so each exp output is a
                # contiguous block covering BOTH heads: each score tile packs
                # (chunk, head) as (plane, 196-slot) and one exp drains it.
                for hp in range(HG // 2):
                    texp2d = texpp.tile([MC, C, 2, NQ], BF16, tag="texp")
                    for ct, cs in enumerate(((0, 1), (2, 3), (4, 5), (6,))):
                        # plane j holds head j's chunks (196-slots) so each
                        # PSUM bank sees matmuls from a single PE row band
                        sc = scp.tile([MC, 2, 512], F32, tag="sc",
                                      name=f"sc{ct}")
                        for si, c in enumerate(cs):
                            for j in range(2):
                                hh = 2 * hp + j
                                pb = 32 * (hh % 4)
                                m2 = hh // 4
                                nc.tensor.matmul(
                                    sc[:, j, 196 * si:196 * si + 196],
                                    lhsT=k_sb[pb:pb + 32, m2, MC * c:MC * c + MC],
                                    rhs=q_sb[pb:pb + 32, m2, b, :],
                                    start=True, stop=True,
                                    tile_position=(pb, 0),
                                    skip_group_check=True)
                        if len(cs) == 2:
                            nc.scalar.activation(
                                texp2d[:, cs[0]:cs[0] + 2, :, :],
                                sc[:, :, 0:392].rearrange(
                                    "p a (b q) -> p b a q", q=196), AF.Exp)
                        else:
                            nc.scalar.activation(
                                texp2d[:, 6, :, :],
                                sc[:, :, 0:196], AF.Exp)
                    nc.vector.tensor_tensor(
                        texp2d[:, 0:4, :, :], texp2d[:, 0:4, :, :],
                        ebias_g[:, 0:4, 2 * hp:2 * hp + 2, :], ALU.mult)
                    nc.vector.tensor_tensor(
                        texp2d[:, 4:7, :, :], texp2d[:, 4:7, :, :],
                        ebias_g[:, 4:7, 2 * hp:2 * hp + 2, :], ALU.mult)

                    # attn @ v (+ denominator row), both heads into one
                    # 2-plane PSUM tile so the den copy covers the pair
                    op = opp.tile([65, 2, NQ], F32, tag="op")
                    for j in range(2):
                        hh = 2 * hp + j
                        for c in range(C):
                            nc.tensor.matmul(op[:, j, :],
                                             lhsT=vtp[:, c, 65 * hh:65 * hh + 65],
                                             rhs=texp2d[:, c, j, :],
                                             start=(c == 0), stop=(c == C - 1))
                    h0 = 8 * g + 2 * hp
                    t = h0 // 2
                    for j in range(2):
                        nc.scalar.activation(acc[b][64 * j:64 * j + 64, t, :],
                                             op[0:64, j, :], AF.Copy)
                    den_dst = den[b][32 * (h0 // 4):32 * (h0 // 4) + 1,
                                     h0 % 4:h0 % 4 + 2, :]
                    if hp % 2 == 0:
                        nc.vector.tensor_copy(den_dst, op[64:65, :, :])
                    else:
                        nc.scalar.copy(den_dst, op[64:65, :, :])

        # ---- output phase ----
        # constants used only here; queued once attention is underway
        nc.sync.dma_start(shv_sb, shv)
        nc.sync.dma_start(sel, seld)
        nc.sync.dma_start(shp_sb, shp)
        nc.sync.dma_start(wp_sb, wp.rearrange("(c p) n -> p c n", p=128))
        for b in range(BC):
            nc.sync.dma_start(
                den2[b],
                den[b].rearrange("(a c) d e -> a c d e", c=32)[:, 0, :, :])
            rec = den2[b]
            nc.vector.reciprocal(rec, den2[b])
            nc.vector.tensor_copy(recb[b], rec)
            hsw = hswp.tile([128, 8, NQ], BF16, tag="hsw")
            for tp in range(4):
                # two t-tiles per round share a PSUM recip-broadcast tile so
                # the DVE chain runs on [128, 392] slabs
                rep = mmp.tile([128, 512], F32, tag="mm", name="rep")
                for ti in range(2):
                    nc.tensor.matmul(rep[:, 196 * ti:196 * ti + 196],
                                     lhsT=sel[:, 2 * tp + ti, :], rhs=recb[b],
                                     start=True, stop=True)
                # v_t = acc*recip + shift ; hsw6 = v_t * clamp(v_t+3,0,6)
                t1 = tmpp.tile([128, 2, NQ], BF16, tag="t1")
                nc.vector.tensor_tensor(
                    t1, acc[b][:, 2 * tp:2 * tp + 2, :],
                    rep[:, 0:392].rearrange("p (a q) -> p a q", q=NQ),
                    ALU.mult)
                vv = tmpp.tile([128, 2, NQ], BF16, tag="vv")
                for ti in range(2):
                    nc.vector.tensor_scalar_add(
                        vv[:, ti, :], t1[:, ti, :],
                        shv_sb[:, 2 * tp + ti:2 * tp + ti + 1])
                t3 = t1
                nc.vector.tensor_scalar(t3, vv, -3.0, 3.0, ALU.max, ALU.min)
                nc.vector.scalar_tensor_tensor(hsw[:, 2 * tp:2 * tp + 2, :],
                                               t3, 3.0, vv,
                                               ALU.add, ALU.mult)
            for mt, msz in ((0, 128), (1, 68)):
                po = mmp.tile([128, 512], F32, tag="mm", name="po")
                nc.tensor.matmul(po[:msz, :], lhsT=ones1[0:1, 0:msz],
                                 rhs=shp_sb, start=True, stop=False,
                                 skip_group_check=True)
                for kk in range(8):
                    nc.tensor.matmul(
                        po[:msz, :],
                        lhsT=hsw[:, kk, 128 * mt:128 * mt + msz],
                        rhs=wp_sb[:, kk, :], start=False,
                        stop=(kk == 7), skip_group_check=True)
                fin = finp.tile([128, OUT], F32, tag="fin")
                nc.scalar.activation(fin[:msz, :], po[:msz, :], AF.Copy)
                nc.sync.dma_start(out[b, 128 * mt:128 * mt + msz, :],
                                  fin[:msz, :])
    nc.compile()
    return nc


def _prepare_in_maps(inputs):
    inp = {k: np.asarray(v) for k, v in inputs.items()}
    x = inp["x"].astype(np.float32)          # [32, 784, 384]
    Wkv, Wq, Wp = inp["Wkv"], inp["Wq"], inp["Wp"]
    biases, idxs = inp["biases"], inp["idxs"].astype(np.int64)

    s_kv = inp["kv_w"] / np.sqrt(inp["kv_var"] + EPS)
    wkv = (Wkv * s_kv[:, None]).astype(np.float32)
    sh_kv = (inp["kv_b"] - inp["kv_mean"] * s_kv).astype(np.float32)
    wkv3 = wkv.reshape(H, KD + D, IN)
    sh3 = sh_kv.reshape(H, KD + D)
    wkT = np.ascontiguousarray(wkv3[:, :KD, :].reshape(H * KD, IN).T).astype(
        ml_dtypes.bfloat16)
    sh_k = np.ascontiguousarray(sh3[:, :KD].reshape(H * KD))
    wvT = np.ascontiguousarray(wkv3[:, KD:, :].reshape(H * D, IN).T).astype(
        ml_dtypes.bfloat16)
    sh_v = np.ascontiguousarray(sh3[:, KD:].reshape(H * D))

    s_q = inp["q_w"] / np.sqrt(inp["q_var"] + EPS)
    wqT = np.ascontiguousarray((Wq * (s_q * SCALE)[:, None]).T).astype(
        ml_dtypes.bfloat16)
    sh_q = ((inp["q_b"] - inp["q_mean"] * s_q) * SCALE).astype(np.float32)

    s_p = inp["p_w"] / np.sqrt(inp["p_var"] + EPS)
    wpT = np.ascontiguousarray(((Wp * s_p[:, None]) / 6.0).T).astype(
        ml_dtypes.bfloat16)
    sh_p = (inp["p_b"] - inp["p_mean"] * s_p).astype(np.float32)

    eb = np.exp(biases.astype(np.float64))[:, idxs]      # [16, 196, 784]
    eb = eb.transpose(0, 2, 1).reshape(H, C, MC, NQ)
    eb = eb.transpose(2, 0, 1, 3).reshape(MC, G, HG, C, NQ)
    eb = np.ascontiguousarray(eb.transpose(0, 1, 3, 2, 4)).astype(
        ml_dtypes.bfloat16)                              # [MC, G, C, HG, NQ]

    xs = x.reshape(B, RES, RES, IN)[:, ::STRIDE, ::STRIDE].reshape(B, NQ, IN)

    shk_h = np.ascontiguousarray(sh_k.reshape(4, 128).T)
    shq_h = np.ascontiguousarray(sh_q.reshape(4, 128).T)
    shv_h = np.ascontiguousarray(sh_v.reshape(8, 128).T)
    shp_h = np.ascontiguousarray(sh_p.reshape(1, OUT))

    sel_h = np.zeros((16, 8, 128), np.float32)
    for t in range(8):
        sel_h[2 * t, t, 0:64] = 1.0
        sel_h[2 * t + 1, t, 64:128] = 1.0
    sel_h = sel_h.astype(ml_dtypes.bfloat16)
    shared = {"wk": wkT, "wv": wvT, "wq": wqT, "wp": wpT, "shk": shk_h,
              "shq": shq_h, "shv": shv_h, "shp": shp_h, "ebias": eb,
              "seld": sel_h}
    in_maps = []
    for i in range(NCORES):
        xb = x[BC * i:BC * i + BC]
        xsb = xs[BC * i:BC * i + BC]
        m = dict(shared)
        m["xT"] = np.ascontiguousarray(xb.transpose(0, 2, 1)).astype(
            ml_dtypes.bfloat16)
        m["xsT"] = np.ascontiguousarray(xsb.transpose(2, 0, 1)).astype(
            ml_dtypes.bfloat16)
        in_maps.append(m)
    return in_maps


def kernel(**inputs):
    global _NC_CACHE, LAST_RESULTS
    in_maps = _prepare_in_maps(inputs)
    if _NC_CACHE is None:
        _NC_CACHE = _build_nc()
    res = run_bass_kernel_spmd(_NC_CACHE, in_maps,
                               core_ids=list(range(NCORES)), trace=TRACE)
    LAST_RESULTS = res
    return np.concatenate([res.results[i]["out"] for i in range(NCORES)],
                          axis=0)
